# revision 1
# baseline (speedup 1.0000x reference)
"""MoE GemmaMLP (top-2 of 8 experts + shared expert) on 8 trn2 NeuronCores.

Sharding: expert-parallel with load balancing.  The host computes top-2
routing from router_logits, chunks each expert's routed batches into pairs,
and packs the pairs into "weight-stream groups" of <=2 pairs sharing one
expert.  The seed-0 load (pairs per expert [5,2,6,4,4,6,4,3] = 34) packs
into exactly 16 two-pair groups + 2 one-pair groups, so every core gets the
SPMD-uniform slot config (2,2,1) = 5 pairs — the integer optimum — with the
group's expert weights passed per-slot as zero-copy views.  Inside a group
the expert weights stream from HBM exactly once while gate/up/down are
interleaved per i-tile (down contracts CHUNK i-tiles in PSUM scratch, then
accumulates into per-pair SBUF output tiles).  The shared expert is
tensor-parallel over the intermediate dim: core c owns columns
[c*512, (c+1)*512) of shared gate/up (matching rows of shared down), kept
resident in SBUF, and processes all 32 batches.  The host sums the per-core
partials and applies the routing weights to the (unweighted) expert rows.
DMAs alternate between the SP and ACT HWDGE rings — a single ring's issue
path is a hard HW bottleneck the cost model misses (2.08ms -> 1.19ms).

Layout trick: x is transposed on the host to xT [B, H, S] so that every
matmul runs transpose-free on the PE:
  gate/up:  g^T[i,s] = sum_h Wg[h,i] * xT[h,s]   (lhsT = Wg tile, rhs = xT)
  down:     out[s,h] = sum_i a^T[i,s] * Wd[i,h]  (lhsT = a^T tile, rhs = Wd)
Batches are processed in pairs (2*S = 512 moving columns) to hit the fp32
moving-operand max and keep the float32r matmul at full rate.
"""

import os
import numpy as np
from contextlib import ExitStack

import concourse.bass as bass
import concourse.mybir as mybir
import concourse.tile as tile
from concourse import bacc
from concourse.bass_utils import run_bass_kernel_spmd

B, S, H, I, E = 32, 256, 1024, 4096, 8
TOP_K = 2
NUM_MOE_LAYERS = 12
NCORES = 8
IS = I // NCORES          # shared-expert intermediate slice per core
HT = H // 128             # h-tiles
P = 128

F32 = mybir.dt.float32
MM_DT = mybir.dt.float32r  # matmul compute dtype (f32r = full-rate, ~tf32)
GELU = mybir.ActivationFunctionType.Gelu_apprx_tanh


def _shared_group(nc, pools, xt_rows, shg, shu, shd, out_rows):
    """Shared-expert MLP (I-slice IS) over a group of 2 pairs (4 batches).

    Weights are resident SBUF tiles:
      shg/shu: [128, HT*IS]  (h-tile t, i-tile i lhsT at cols t*IS + i*128)
      shd:     [128, (IS/128)*H]  (i-tile i rhs at cols i*H)
    """
    xt_p, psgu, pssc, tmp_p, at_p, st_p = (
        pools[k] for k in ("xt", "psgu", "pssc", "tmp", "aT", "stage"))
    S2 = 2 * S
    NI = IS // P
    NP = 2

    xt_t = []
    for pr in range(NP):
        t = xt_p.tile([P, HT * S2], MM_DT, tag="xt", name=f"xts{pr}")
        v = t[:].rearrange("p (t c) -> p t c", c=S2)
        nc.sync.dma_start(
            v[:, :, 0:S],
            xt_rows[2 * pr].rearrange("(t p) s -> p t s", p=P).bitcast(MM_DT))
        nc.scalar.dma_start(
            v[:, :, S:S2],
            xt_rows[2 * pr + 1].rearrange("(t p) s -> p t s", p=P)
            .bitcast(MM_DT))
        xt_t.append(t)

    at_t = at_p.tile([P, NI * NP * S2], MM_DT, tag="aT")
    for i in range(NI):
        ps_g = [psgu.tile([P, S2], F32, tag="ps", name=f"spsg{pr}")
                for pr in range(NP)]
        ps_u = [psgu.tile([P, S2], F32, tag="ps", name=f"spsu{pr}")
                for pr in range(NP)]
        for t in range(HT):
            for pr in range(NP):
                nc.tensor.matmul(ps_g[pr][:],
                                 shg[:, t * IS + i * P: t * IS + (i + 1) * P],
                                 xt_t[pr][:, t * S2:(t + 1) * S2],
                                 start=(t == 0), stop=(t == HT - 1))
        for t in range(HT):
            for pr in range(NP):
                nc.tensor.matmul(ps_u[pr][:],
                                 shu[:, t * IS + i * P: t * IS + (i + 1) * P],
                                 xt_t[pr][:, t * S2:(t + 1) * S2],
                                 start=(t == 0), stop=(t == HT - 1))
        for pr in range(NP):
            tmp_g = tmp_p.tile([P, S2], F32, tag="tmp")
            nc.scalar.activation(tmp_g[:], ps_g[pr][:], GELU)
            col = (i * NP + pr) * S2
            nc.vector.tensor_mul(at_t[:, col:col + S2], tmp_g[:],
                                 ps_u[pr][:])

    for pr in range(NP):
        for ss in range(4):
            st = st_p.tile([P, H], F32, tag="stage")
            for hg in range(2):
                sc = pssc.tile([P, 512], F32, tag="sc")
                for i in range(NI):
                    col = (i * NP + pr) * S2 + ss * P
                    nc.tensor.matmul(sc[:], at_t[:, col:col + P],
                                     shd[:, i * H + hg * 512:
                                         i * H + (hg + 1) * 512],
                                     start=(i == 0), stop=(i == NI - 1))
                nc.vector.tensor_copy(st[:, hg * 512:(hg + 1) * 512], sc[:])
            b = 2 * pr + (ss // 2)
            s0 = (ss % 2) * P
            nc.sync.dma_start(out_rows[b][s0:s0 + P, :], st[:])


CHUNK = 4                 # i-tiles per down-accumulation chunk
GROUPS = ((2, 32), (2, 32), (1, 4), (1, 4))  # (pairs, i-tiles) per slot


def _expert_group(nc, pools, xt_rows, wg_d, wu_d, wd_d, out_rows, n_pairs,
                  ni=I // P):
    """One weight-stream group: n_pairs pairs sharing one expert's weights.

    Interleaved structure: per i-tile, gate/up matmuls for all pairs build
    a^T for a CHUNK of i-tiles; the down projection then contracts that
    chunk into PSUM scratch and accumulates into per-pair SBUF output
    accumulators, so weights stream exactly once per group.

    xt_rows: list of 2*n_pairs DRAM APs [H, S]
    out_rows: list of 2*n_pairs DRAM APs [S, H]
    """
    xt_p, psgu, pssc, tmp_p, at_p, ob_p = (
        pools[k] for k in ("xt", "psgu", "pssc", "tmp", "aT", "outsb"))
    S2 = 2 * S
    NI = ni

    xt_t = []
    for pr in range(n_pairs):
        t = xt_p.tile([P, HT * S2], MM_DT, tag="xt", name=f"xt{pr}")
        v = t[:].rearrange("p (t c) -> p t c", c=S2)
        nc.sync.dma_start(
            v[:, :, 0:S],
            xt_rows[2 * pr].rearrange("(t p) s -> p t s", p=P).bitcast(MM_DT))
        nc.scalar.dma_start(
            v[:, :, S:S2],
            xt_rows[2 * pr + 1].rearrange("(t p) s -> p t s", p=P)
            .bitcast(MM_DT))
        xt_t.append(t)

    # per-pair output accumulators [128 s, (ss, hg) * 512]
    out_sb = [ob_p.tile([P, 4 * 2 * 512], F32, tag="outsb",
                        name=f"osb{pr}") for pr in range(n_pairs)]

    for c0 in range(0, NI, CHUNK):
        chunk = range(c0, min(c0 + CHUNK, NI))
        ch_n = len(chunk)
        at_t = at_p.tile([P, CHUNK * n_pairs * S2], MM_DT, tag="aT")
        wd_ts = []
        for ci, i in enumerate(chunk):
            wg_t = pools["wg"].tile([P, HT * P], MM_DT, tag="wg")
            nc.sync.dma_start(
                wg_t[:].rearrange("p (t i) -> p t i", i=P),
                wg_d.rearrange("(t p) i -> p t i", p=P)
                [:, :, i * P:(i + 1) * P].bitcast(MM_DT))
            wu_t = pools["wu"].tile([P, HT * P], MM_DT, tag="wu")
            nc.scalar.dma_start(
                wu_t[:].rearrange("p (t i) -> p t i", i=P),
                wu_d.rearrange("(t p) i -> p t i", p=P)
                [:, :, i * P:(i + 1) * P].bitcast(MM_DT))
            ps_g = [psgu.tile([P, S2], F32, tag="ps", name=f"psg{pr}")
                    for pr in range(n_pairs)]
            ps_u = [psgu.tile([P, S2], F32, tag="ps", name=f"psu{pr}")
                    for pr in range(n_pairs)]
            for t in range(HT):
                for pr in range(n_pairs):
                    nc.tensor.matmul(ps_g[pr][:], wg_t[:, t * P:(t + 1) * P],
                                     xt_t[pr][:, t * S2:(t + 1) * S2],
                                     start=(t == 0), stop=(t == HT - 1))
            for t in range(HT):
                for pr in range(n_pairs):
                    nc.tensor.matmul(ps_u[pr][:], wu_t[:, t * P:(t + 1) * P],
                                     xt_t[pr][:, t * S2:(t + 1) * S2],
                                     start=(t == 0), stop=(t == HT - 1))
            wd_t = pools["wd"].tile([P, H], MM_DT, tag="wd")
            nc.sync.dma_start(wd_t[:],
                              wd_d[i * P:(i + 1) * P, :].bitcast(MM_DT))
            wd_ts.append(wd_t)
            for pr in range(n_pairs):
                tmp_g = tmp_p.tile([P, S2], F32, tag="tmp")
                nc.scalar.activation(tmp_g[:], ps_g[pr][:], GELU)
                col = (ci * n_pairs + pr) * S2
                nc.vector.tensor_mul(at_t[:, col:col + S2], tmp_g[:],
                                     ps_u[pr][:])

        # down for this chunk: accumulate into out_sb
        for pr in range(n_pairs):
            for ss in range(4):
                for hg in range(2):
                    sc = pssc.tile([P, 512], F32, tag="sc")
                    for ci in range(ch_n):
                        col = (ci * n_pairs + pr) * S2 + ss * P
                        nc.tensor.matmul(sc[:], at_t[:, col:col + P],
                                         wd_ts[ci][:, hg * 512:(hg + 1) * 512],
                                         start=(ci == 0), stop=(ci == ch_n - 1))
                    dst = out_sb[pr][:, (ss * 2 + hg) * 512:
                                     (ss * 2 + hg + 1) * 512]
                    if c0 == 0:
                        nc.vector.tensor_copy(dst, sc[:])
                    else:
                        nc.vector.tensor_add(dst, dst, sc[:])

    for pr in range(n_pairs):
        for ss in range(4):
            b = 2 * pr + (ss // 2)
            s0 = (ss % 2) * P
            nc.sync.dma_start(out_rows[b][s0:s0 + P, :],
                              out_sb[pr][:, ss * H:(ss + 1) * H])


def _build_kernel(C, nreps=1):
    """C = per-core routed-batch capacity (= 2 * sum of slot pairs)."""
    assert C == 2 * sum(np_ for np_, _ in GROUPS)
    nc = bacc.Bacc("TRN2", target_bir_lowering=False, debug=False,
                   num_devices=NCORES)
    xt_r = nc.dram_tensor("xt_r", [C, H, S], F32, kind="ExternalInput").ap()
    xt_all = nc.dram_tensor("xt_all", [B, H, S], F32, kind="ExternalInput").ap()
    wexp = []
    for gi, (np_, ni_) in enumerate(GROUPS):
        wi = ni_ * P
        wexp.append(tuple(
            nc.dram_tensor(f"w{nm}_{gi}", shp, F32, kind="ExternalInput").ap()
            for nm, shp in (("g", [H, wi]), ("u", [H, wi]), ("d", [wi, H]))))
    wg_s = nc.dram_tensor("wg_s", [H, IS], F32, kind="ExternalInput").ap()
    wu_s = nc.dram_tensor("wu_s", [H, IS], F32, kind="ExternalInput").ap()
    wd_s = nc.dram_tensor("wd_s", [IS, H], F32, kind="ExternalInput").ap()
    out_r = nc.dram_tensor("out_r", [C, S, H], F32, kind="ExternalOutput").ap()
    out_s = nc.dram_tensor("out_s", [B, S, H], F32, kind="ExternalOutput").ap()

    with tile.TileContext(nc) as tc, ExitStack() as ctx:
        pools = {
            "xt": ctx.enter_context(tc.tile_pool(name="xt", bufs=2)),
            "psgu": ctx.enter_context(
                tc.tile_pool(name="psgu", bufs=7, space="PSUM")),
            "pssc": ctx.enter_context(
                tc.tile_pool(name="pssc", bufs=1, space="PSUM")),
            "psum": None,  # set below: shared-phase pools alias psgu/pssc
            "tmp": ctx.enter_context(tc.tile_pool(name="tmp", bufs=2)),
            "aT": ctx.enter_context(tc.tile_pool(name="aT", bufs=2)),
            "outsb": ctx.enter_context(tc.tile_pool(name="outsb", bufs=2)),
            "stage": ctx.enter_context(tc.tile_pool(name="stage", bufs=2)),
            "wg": ctx.enter_context(tc.tile_pool(name="wg", bufs=2)),
            "wu": ctx.enter_context(tc.tile_pool(name="wu", bufs=2)),
            "wd": ctx.enter_context(tc.tile_pool(name="wd", bufs=CHUNK)),
            "shw": ctx.enter_context(tc.tile_pool(name="shw", bufs=1)),
        }
        pools["psum"] = pools["psgu"]

        # ---- expert phase: weight-stream groups ---------------------------
        for _rep in range(nreps):
            row = 0
            for gi, (npair, ni_) in enumerate(GROUPS):
                rows = list(range(row, row + 2 * npair))
                _expert_group(nc, pools,
                              [xt_r[r] for r in rows],
                              wexp[gi][0], wexp[gi][1], wexp[gi][2],
                              [out_r[r] for r in rows], npair, ni=ni_)
                row += 2 * npair

        # ---- shared phase: all batches, resident weight slice -------------
        shg = pools["shw"].tile([P, HT * IS], MM_DT, tag="shg")
        shu = pools["shw"].tile([P, HT * IS], MM_DT, tag="shu")
        shd = pools["shw"].tile([P, (IS // P) * H], MM_DT, tag="shd")
        nc.sync.dma_start(
            shg[:].rearrange("p (t c) -> p t c", c=IS),
            wg_s.rearrange("(t p) c -> p t c", p=P).bitcast(MM_DT))
        nc.scalar.dma_start(
            shu[:].rearrange("p (t c) -> p t c", c=IS),
            wu_s.rearrange("(t p) c -> p t c", p=P).bitcast(MM_DT))
        nc.sync.dma_start(
            shd[:].rearrange("p (ib h) -> p ib h", h=H),
            wd_s.rearrange("(ib p) h -> p ib h", p=P).bitcast(MM_DT))

        for _rep in range(nreps):
            for g in range(B // 4):
                rows = list(range(4 * g, 4 * g + 4))
                _shared_group(nc, pools, [xt_all[r] for r in rows],
                              shg[:], shu[:], shd[:],
                              [out_s[r] for r in rows])

    nc.compile()
    return nc


_KERNEL_CACHE = {}


def _get_kernel(groups):
    if groups not in _KERNEL_CACHE:
        global GROUPS
        GROUPS = groups
        _KERNEL_CACHE[groups] = _build_kernel(
            2 * sum(np_ for np_, _ in groups))
    return _KERNEL_CACHE[groups]


def _routing(router_logits):
    """Replicate reference routing in numpy f32: softmax, top-2, renorm."""
    rl = np.asarray(router_logits, np.float32)
    m = rl.max(axis=-1, keepdims=True)
    ex = np.exp(rl - m, dtype=np.float32)
    rw = ex / ex.sum(axis=-1, keepdims=True)
    sel = np.argsort(-rw, axis=-1, kind="stable")[:, :TOP_K]
    w = np.take_along_axis(rw, sel, axis=-1)
    w = w / w.sum(axis=-1, keepdims=True)
    scale = np.float32(1.0 / NUM_MOE_LAYERS)
    w = scale * w + (np.float32(1.0) - scale) * w
    return sel, w.astype(np.float32)


def kernel(x, router_logits, skill_gate, skill_up, skill_down,
           shared_gate, shared_up, shared_down):
    x = np.asarray(x, np.float32)
    skill_gate = np.asarray(skill_gate, np.float32)
    skill_up = np.asarray(skill_up, np.float32)
    skill_down = np.asarray(skill_down, np.float32)
    shared_gate = np.asarray(shared_gate, np.float32)
    shared_up = np.asarray(shared_up, np.float32)
    shared_down = np.asarray(shared_down, np.float32)

    sel, w = _routing(router_logits)
    lists = [[] for _ in range(E)]
    wmap = np.zeros((B, E), np.float32)
    for b in range(B):
        for k in range(TOP_K):
            e = int(sel[b, k])
            lists[e].append(b)
            wmap[b, e] = w[b, k]

    # decompose each expert's routed batches into weight-stream groups of
    # <=2 pairs; entries are (batch, is_real).  Two-pair groups are assigned
    # to one core each ("own" slots); leftover single pairs become
    # tensor-parallel slots split over I across ALL cores.
    groups2, groups1 = [], []
    for e in range(E):
        ent = [(b, True) for b in lists[e]]
        if len(ent) % 2:
            ent.append((0, False))
        pairs = [ent[i:i + 2] for i in range(0, len(ent), 2)]
        for i in range(0, len(pairs) - 1, 2):
            groups2.append((e, pairs[i] + pairs[i + 1]))
        if len(pairs) % 2:
            groups1.append((e, pairs[-1]))
    n2 = max(1, -(-len(groups2) // NCORES))
    n_tp = len(groups1)
    TPI = I // NCORES  # i-columns per core for a tp slot
    cfg = ((2, I // P),) * n2 + ((1, TPI // P),) * n_tp
    dummy2 = (0, [(0, False)] * 4)
    groups2 += [dummy2] * (n2 * NCORES - len(groups2))

    xt = np.ascontiguousarray(x.transpose(0, 2, 1))  # [B, H, S]
    nc = _get_kernel(cfg)

    in_maps = []
    core_slots = []
    for c in range(NCORES):
        own = [groups2[c * n2 + j] for j in range(n2)]
        core_slots.append(own)
        batches = [b for _, ent in own for b, _ in ent]
        batches += [b for _, ent in groups1 for b, _ in ent]
        m = {
            "xt_r": np.ascontiguousarray(xt[batches]),
            "xt_all": xt,
            "wg_s": np.ascontiguousarray(shared_gate[:, c * IS:(c + 1) * IS]),
            "wu_s": np.ascontiguousarray(shared_up[:, c * IS:(c + 1) * IS]),
            "wd_s": np.ascontiguousarray(shared_down[c * IS:(c + 1) * IS, :]),
        }
        for gi, (e, _) in enumerate(own):
            m[f"wg_{gi}"] = skill_gate[e]
            m[f"wu_{gi}"] = skill_up[e]
            m[f"wd_{gi}"] = skill_down[e]
        for tj, (e, _) in enumerate(groups1):
            gi = n2 + tj
            sl = slice(c * TPI, (c + 1) * TPI)
            m[f"wg_{gi}"] = np.ascontiguousarray(skill_gate[e][:, sl])
            m[f"wu_{gi}"] = np.ascontiguousarray(skill_up[e][:, sl])
            m[f"wd_{gi}"] = np.ascontiguousarray(skill_down[e][sl, :])
        in_maps.append(m)

    trace = bool(os.environ.get("TRNK_TRACE"))
    res = run_bass_kernel_spmd(nc, in_maps, core_ids=list(range(NCORES)),
                               trace=trace,
                               trace_cores=list(range(NCORES)) if trace else None)
    kernel.last_exec_time_ns = res.exec_time_ns
    kernel.last_results = res
    kernel.last_nc = nc
    kernel.last_in_maps = in_maps

    out = np.zeros((B, S, H), np.float32)
    n_own_rows = 0
    for c in range(NCORES):
        r = res.results[c]["out_r"]
        row = 0
        for e, ent in core_slots[c]:
            for b, real in ent:
                if real:
                    out[b] += wmap[b, e] * r[row]
                row += 1
        n_own_rows = row
    # tp slots: rows are partial (I-slice) sums — reduce across cores
    for tj, (e, ent) in enumerate(groups1):
        for k, (b, real) in enumerate(ent):
            if real:
                row = n_own_rows + 2 * tj + k
                part = sum(res.results[c]["out_r"][row]
                           for c in range(NCORES))
                out[b] += wmap[b, e] * part
    for c in range(NCORES):
        out += res.results[c]["out_s"]
    return out



# revision 3
# speedup vs baseline: 3.5578x; 3.5578x over previous
"""MoE GemmaMLP (top-2 of 8 experts + shared expert) on 8 trn2 NeuronCores.

v2: bf16 + host-packed contiguous DMA + shared expert folded in as a
data-parallel group.

Sharding: expert-parallel with load balancing.  The host computes top-2
routing from router_logits, chunks each expert's routed batches into pairs,
and packs pairs into weight-stream groups of <=2 pairs sharing one expert's
weights.  The seed-0 load (34 pairs) packs into 16 two-pair groups + 2
single pairs, so every core gets the SPMD-uniform slot config: two 2-pair
expert groups, one 2-pair shared-expert group (data-parallel: core c owns
batches 4c..4c+3, full I), and two tensor-parallel single-pair slots
(leftover pairs, I/8 slice per core, host-reduced).

Everything the device touches is bf16 and packed on the host into the exact
SBUF tile layout, so every DMA is a plain [128, 4096] contiguous copy (128
descriptors x 8KB) — minimizing HWDGE issue-path pressure, which is the
dominant HW overhead the cost model misses.  DMAs alternate between the SP
and ACT HWDGE rings.

Per group: expert weights stream from HBM exactly once, chunked 4 i-tiles
at a time; gate/up matmuls (stationary = weight tile, moving = 2 batches x
256 = 512 columns) fill PSUM, gelu*up produces a^T in bf16, and the down
projection contracts each chunk with hg-interleaved PSUM chains (stationary
a^T chunk loaded once for both h-halves), accumulating into per-pair f32
SBUF tiles; a final bf16 staging copy is DMA'd to a packed output that the
host unpacks, weights (routing), and reduces.
"""

import math
import numpy as np
from contextlib import ExitStack

import concourse.bass as bass
import concourse.mybir as mybir
import concourse.tile as tile
from concourse import bacc
from concourse.bass_utils import run_bass_kernel_spmd

B, S, H, I, E = 32, 256, 1024, 4096, 8
TOP_K = 2
NUM_MOE_LAYERS = 12
NCORES = 8
P = 128
HT = H // P               # 8 h-tiles
S2 = 2 * S                # 512 moving columns (one pair)
CHUNK = 4                 # i-tiles per weight-stream / down-accum chunk
NIT = I // P              # 32 i-tiles for a full expert
TPI = I // NCORES         # 512 i-cols per core for a tp slot (= CHUNK*P)

F32 = mybir.dt.float32
BF16 = mybir.dt.bfloat16
NPBF = mybir.dt.np(BF16)
GELU = mybir.ActivationFunctionType.Gelu_apprx_tanh


def _group(nc, pools, ring, xtp, wgp, wup, wdp, outp, npair, ni):
    """One weight-stream group: npair pairs sharing one expert's weights."""
    nch = ni // CHUNK
    CW = CHUNK * P        # 512 i-cols per chunk

    xt_t = []
    for pr in range(npair):
        t = pools["xt"].tile([P, HT * S2], BF16, tag="xt", name=f"xt{pr}")
        ring()(t[:], xtp[pr])
        xt_t.append(t)
    out_sb = [pools["outsb"].tile([P, 8 * 512], F32, tag="outsb",
                                  name=f"osb{pr}")
              for pr in range(npair)]

    for c in range(nch):
        wg_t = pools["wg"].tile([P, HT * CW], BF16, tag="wg")
        ring()(wg_t[:], wgp[c])
        wu_t = pools["wu"].tile([P, HT * CW], BF16, tag="wu")
        ring()(wu_t[:], wup[c])
        wd_t = pools["wd"].tile([P, CHUNK * H], BF16, tag="wd")
        ring()(wd_t[:], wdp[c])
        at_t = pools["aT"].tile([P, CHUNK * npair * S2], BF16, tag="aT")

        for it in range(CHUNK):
            ps_g = [pools["psgu"].tile([P, S2], F32, tag="ps",
                                       name=f"psg{pr}")
                    for pr in range(npair)]
            ps_u = [pools["psgu"].tile([P, S2], F32, tag="ps",
                                       name=f"psu{pr}")
                    for pr in range(npair)]
            for t in range(HT):
                col = t * CW + it * P
                for pr in range(npair):
                    nc.tensor.matmul(ps_g[pr][:], wg_t[:, col:col + P],
                                     xt_t[pr][:, t * S2:(t + 1) * S2],
                                     start=(t == 0), stop=(t == HT - 1))
            for t in range(HT):
                col = t * CW + it * P
                for pr in range(npair):
                    nc.tensor.matmul(ps_u[pr][:], wu_t[:, col:col + P],
                                     xt_t[pr][:, t * S2:(t + 1) * S2],
                                     start=(t == 0), stop=(t == HT - 1))
            for pr in range(npair):
                tmp = pools["tmp"].tile([P, S2], F32, tag="tmp")
                nc.scalar.activation(tmp[:], ps_g[pr][:], GELU)
                acol = (it * npair + pr) * S2
                nc.vector.tensor_mul(at_t[:, acol:acol + S2], tmp[:],
                                     ps_u[pr][:])

        for pr in range(npair):
            for ss in range(4):
                sc = [pools["pssc"].tile([P, 512], F32, tag="sc",
                                         name=f"sc{hg}")
                      for hg in range(2)]
                for ci in range(CHUNK):
                    acol = (ci * npair + pr) * S2 + ss * P
                    for hg in range(2):
                        nc.tensor.matmul(
                            sc[hg][:], at_t[:, acol:acol + P],
                            wd_t[:, ci * H + hg * 512:ci * H + (hg + 1) * 512],
                            start=(ci == 0), stop=(ci == CHUNK - 1))
                for hg in range(2):
                    dst = out_sb[pr][:, (ss * 2 + hg) * 512:
                                     (ss * 2 + hg + 1) * 512]
                    if c == 0:
                        nc.vector.tensor_copy(dst, sc[hg][:])
                    else:
                        nc.vector.tensor_add(dst, dst, sc[hg][:])

    for pr in range(npair):
        st = pools["stage"].tile([P, 8 * 512], BF16, tag="stage")
        nc.vector.tensor_copy(st[:], out_sb[pr][:])
        ring()(outp[pr], st[:])


def _build_kernel(groups):
    """groups: tuple of (n_pairs, n_itiles) per weight-stream slot."""
    nc = bacc.Bacc("TRN2", target_bir_lowering=False, debug=False,
                   num_devices=NCORES)
    g_t = []
    for gi, (npair, ni) in enumerate(groups):
        nch = ni // CHUNK
        xtp = nc.dram_tensor(f"xt_{gi}", [npair, P, HT * S2], BF16,
                             kind="ExternalInput").ap()
        wgp = nc.dram_tensor(f"wg_{gi}", [nch, P, HT * CHUNK * P], BF16,
                             kind="ExternalInput").ap()
        wup = nc.dram_tensor(f"wu_{gi}", [nch, P, HT * CHUNK * P], BF16,
                             kind="ExternalInput").ap()
        wdp = nc.dram_tensor(f"wd_{gi}", [nch, P, CHUNK * H], BF16,
                             kind="ExternalInput").ap()
        outp = nc.dram_tensor(f"out_{gi}", [npair, P, 8 * 512], BF16,
                              kind="ExternalOutput").ap()
        g_t.append((xtp, wgp, wup, wdp, outp))

    with tile.TileContext(nc) as tc, ExitStack() as ctx:
        pools = {
            "xt": ctx.enter_context(tc.tile_pool(name="xt", bufs=4)),
            "psgu": ctx.enter_context(
                tc.tile_pool(name="psgu", bufs=5, space="PSUM")),
            "pssc": ctx.enter_context(
                tc.tile_pool(name="pssc", bufs=3, space="PSUM")),
            "tmp": ctx.enter_context(tc.tile_pool(name="tmp", bufs=3)),
            "aT": ctx.enter_context(tc.tile_pool(name="aT", bufs=2)),
            "outsb": ctx.enter_context(tc.tile_pool(name="outsb", bufs=3)),
            "stage": ctx.enter_context(tc.tile_pool(name="stage", bufs=2)),
            "wg": ctx.enter_context(tc.tile_pool(name="wg", bufs=2)),
            "wu": ctx.enter_context(tc.tile_pool(name="wu", bufs=2)),
            "wd": ctx.enter_context(tc.tile_pool(name="wd", bufs=2)),
        }
        rng = {"i": 0}

        def ring():
            eng = nc.sync if rng["i"] % 2 == 0 else nc.scalar
            rng["i"] += 1
            return eng.dma_start

        for gi, (npair, ni) in enumerate(groups):
            xtp, wgp, wup, wdp, outp = g_t[gi]
            _group(nc, pools, ring, xtp, wgp, wup, wdp, outp, npair, ni)

    nc.compile()
    return nc


_KERNEL_CACHE = {}


def _get_kernel(groups):
    if groups not in _KERNEL_CACHE:
        _KERNEL_CACHE[groups] = _build_kernel(groups)
    return _KERNEL_CACHE[groups]


def _routing(router_logits):
    """Replicate reference routing in numpy f32: softmax, top-2, renorm."""
    rl = np.asarray(router_logits, np.float32)
    m = rl.max(axis=-1, keepdims=True)
    ex = np.exp(rl - m, dtype=np.float32)
    rw = ex / ex.sum(axis=-1, keepdims=True)
    sel = np.argsort(-rw, axis=-1, kind="stable")[:, :TOP_K]
    w = np.take_along_axis(rw, sel, axis=-1)
    w = w / w.sum(axis=-1, keepdims=True)
    return sel, w.astype(np.float32)


def _pack_gu(w):
    """[H, Ic] f32 -> [nch, 128, HT*CHUNK*128] bf16 (chunk, p, t, it, ii)."""
    Ic = w.shape[1]
    nch = Ic // (CHUNK * P)
    return np.ascontiguousarray(
        w.astype(NPBF).reshape(HT, P, nch, CHUNK * P)
        .transpose(2, 1, 0, 3).reshape(nch, P, HT * CHUNK * P))


def _pack_d(wd):
    """[Ir, H] f32 -> [nch, 128, CHUNK*H] bf16 (chunk, p, ci, h)."""
    Ir = wd.shape[0]
    nch = Ir // (CHUNK * P)
    return np.ascontiguousarray(
        wd.astype(NPBF).reshape(nch, CHUNK, P, H)
        .transpose(0, 2, 1, 3).reshape(nch, P, CHUNK * H))


def _pack_x_pair(xb):
    """[2, S, H] f32 -> [128, HT*2*S] bf16 (p, t, b, s)."""
    return np.ascontiguousarray(
        xb.astype(NPBF).reshape(2, S, HT, P)
        .transpose(3, 2, 0, 1).reshape(P, HT * 2 * S))


def _unpack_out(r):
    """[npair, 128, 4096] bf16 -> [npair, 2, S, H] f32 (pair, b, s, h)."""
    npair = r.shape[0]
    return (r.astype(np.float32)
            .reshape(npair, P, 2, 2, 2, 512)      # p, b2, sblk, hg, hh
            .transpose(0, 2, 3, 1, 4, 5)
            .reshape(npair, 2, S, H))


def kernel(x, router_logits, skill_gate, skill_up, skill_down,
           shared_gate, shared_up, shared_down):
    x = np.asarray(x, np.float32)

    sel, w = _routing(router_logits)
    lists = [[] for _ in range(E)]
    wmap = np.zeros((B, E), np.float32)
    for b in range(B):
        for k in range(TOP_K):
            e = int(sel[b, k])
            lists[e].append(b)
            wmap[b, e] = w[b, k]

    # decompose each expert's routed batches into 2-pair weight-stream
    # groups; leftover single pairs become tensor-parallel slots split over
    # I across ALL cores.  Entries are (batch, is_real).
    groups2, groups1 = [], []
    for e in range(E):
        ent = [(b, True) for b in lists[e]]
        if len(ent) % 2:
            ent.append((0, False))
        pairs = [ent[i:i + 2] for i in range(0, len(ent), 2)]
        for i in range(0, len(pairs) - 1, 2):
            groups2.append((e, pairs[i] + pairs[i + 1]))
        if len(pairs) % 2:
            groups1.append((e, pairs[-1]))
    n2 = max(1, -(-len(groups2) // NCORES))
    n_tp = len(groups1)
    dummy2 = (0, [(0, False)] * 4)
    groups2 += [dummy2] * (n2 * NCORES - len(groups2))
    cfg = ((2, NIT),) * n2 + ((2, NIT),) + ((1, CHUNK),) * n_tp
    n_shared_gi = n2            # group index of the shared-expert group

    nc = _get_kernel(cfg)

    # pack weights once (bf16, SBUF layout)
    pg = [_pack_gu(np.asarray(skill_gate[e], np.float32)) for e in range(E)]
    pu = [_pack_gu(np.asarray(skill_up[e], np.float32)) for e in range(E)]
    pd = [_pack_d(np.asarray(skill_down[e], np.float32)) for e in range(E)]
    psg = _pack_gu(np.asarray(shared_gate, np.float32))
    psu = _pack_gu(np.asarray(shared_up, np.float32))
    psd = _pack_d(np.asarray(shared_down, np.float32))

    in_maps = []
    core_groups = []
    for c in range(NCORES):
        own = [groups2[c * n2 + j] for j in range(n2)]
        core_groups.append(own)
        m = {}
        for gi, (e, ent) in enumerate(own):
            m[f"xt_{gi}"] = np.stack([
                _pack_x_pair(x[[ent[0][0], ent[1][0]]]),
                _pack_x_pair(x[[ent[2][0], ent[3][0]]])])
            m[f"wg_{gi}"] = pg[e]
            m[f"wu_{gi}"] = pu[e]
            m[f"wd_{gi}"] = pd[e]
        # shared group: batches 4c..4c+3, full I
        sb = [4 * c + j for j in range(4)]
        m[f"xt_{n_shared_gi}"] = np.stack([
            _pack_x_pair(x[sb[0:2]]), _pack_x_pair(x[sb[2:4]])])
        m[f"wg_{n_shared_gi}"] = psg
        m[f"wu_{n_shared_gi}"] = psu
        m[f"wd_{n_shared_gi}"] = psd
        # tp slots: chunk c of expert e's packed weights is exactly this
        # core's i-slice [c*512, (c+1)*512)
        for tj, (e, ent) in enumerate(groups1):
            gi = n_shared_gi + 1 + tj
            m[f"xt_{gi}"] = _pack_x_pair(
                x[[ent[0][0], ent[1][0]]])[None]
            m[f"wg_{gi}"] = pg[e][c:c + 1]
            m[f"wu_{gi}"] = pu[e][c:c + 1]
            m[f"wd_{gi}"] = pd[e][c:c + 1]
        in_maps.append(m)

    res = run_bass_kernel_spmd(nc, in_maps, core_ids=list(range(NCORES)))
    kernel.last_exec_time_ns = res.exec_time_ns
    kernel.last_results = res
    kernel.last_nc = nc
    kernel.last_in_maps = in_maps

    out = np.zeros((B, S, H), np.float32)
    for c in range(NCORES):
        r = res.results[c]
        for gi, (e, ent) in enumerate(core_groups[c]):
            arr = _unpack_out(r[f"out_{gi}"])
            for pr in range(2):
                for b2 in range(2):
                    b, real = ent[2 * pr + b2]
                    if real:
                        out[b] += wmap[b, e] * arr[pr, b2]
        arr = _unpack_out(r[f"out_{n_shared_gi}"])
        for pr in range(2):
            for b2 in range(2):
                out[4 * c + 2 * pr + b2] += arr[pr, b2]
    for tj, (e, ent) in enumerate(groups1):
        gi = n_shared_gi + 1 + tj
        acc = sum(_unpack_out(res.results[c][f"out_{gi}"])
                  for c in range(NCORES))
        for b2 in range(2):
            b, real = ent[b2]
            if real:
                out[b] += wmap[b, e] * acc[0, b2]
    return out


# revision 9
# speedup vs baseline: 14.6174x; 4.1086x over previous
"""MoE GemmaMLP (top-2 of 8 experts + shared expert) on 8 trn2 NeuronCores.

v4: bf16, host-packed contiguous DMA, shared expert folded in as a
data-parallel group, zero-padding load balance with half-pair slots.

Sharding: expert-parallel with load balancing.  The host computes top-2
routing from router_logits and decomposes each expert's routed batches into
full pairs (2 batches, 512 moving columns) and at most one half pair
(1 batch, 256 moving columns).  These are packed into weight-stream groups
so that every core gets the identical (SPMD-uniform) slot config with no
padded dummy batches:
  - n22 two-pair expert groups + ns single-pair expert groups (full I),
  - one two-pair shared-expert group (data-parallel: core c owns batches
    4c..4c+3, full I),
  - leftover full/half pairs as tensor-parallel slots (I/8 slice per core,
    host-reduced across cores).

Everything the device touches is bf16 and packed on the host into the exact
SBUF tile layout, so every DMA is a plain contiguous [128, <=4096] copy
(128 descriptors x <=8KB) — minimizing HWDGE issue-path pressure, the
dominant HW overhead.  DMAs alternate between the SP and ACT HWDGE rings;
the first chunk's weights are interleaved with the x loads so the PE can
start after ~2 transfers.

Per group: expert weights stream from HBM exactly once, chunked 4 i-tiles
at a time; gate/up matmuls (stationary = weight tile, moving = x columns)
fill PSUM, gelu*up produces a^T in bf16, and the down projection contracts
each chunk with hg-interleaved PSUM chains (stationary a^T block loaded
once for both h-halves), accumulating into per-pair f32 SBUF tiles; the
last chunk's accumulation writes a bf16 staging tile directly, which is
DMA'd per pair as soon as it completes to a packed output blob the host
unpacks, weights (routing), and reduces.
"""

import numpy as np
from contextlib import ExitStack

import concourse.bass as bass
import concourse.mybir as mybir
import concourse.tile as tile
from concourse import bacc
from concourse.bass_utils import run_bass_kernel_spmd

B, S, H, I, E = 32, 256, 1024, 4096, 8
TOP_K = 2
NCORES = 8
P = 128
HT = H // P               # 8 h-tiles
S2 = 2 * S                # 512 moving columns (one full pair)
CHUNK = 4                 # i-tiles per weight-stream / down-accum chunk
NIT = I // P              # 32 i-tiles for a full expert

F32 = mybir.dt.float32
BF16 = mybir.dt.bfloat16
NPBF = mybir.dt.np(BF16)
GELU = mybir.ActivationFunctionType.Gelu_apprx_tanh


def _group(nc, pools, ring, inb, off, outb, sizes, ni):
    """One weight-stream group: pairs of `sizes` batches (2=full, 1=half)
    sharing one expert's weights over `ni` i-tiles."""
    npair = len(sizes)
    scols = [s * S for s in sizes]
    sumsc = sum(scols)
    pre = [sum(scols[:i]) for i in range(npair)]
    nch = ni // CHUNK
    CW = CHUNK * P        # 512 i-cols per chunk

    xt_t = [pools["xt"].tile([P, HT * S2], BF16, tag="xt", name=f"xt{pr}")
            for pr in range(npair)]
    out_sb = [pools["outsb"].tile([P, 8 * 512], F32, tag="outsb",
                                  name=f"osb{pr}")
              for pr in range(npair)] if nch > 1 else [None] * npair
    st_t = [pools["stage"].tile([P, 8 * 512], BF16, tag="stage",
                                name=f"st{pr}")
            for pr in range(npair)]

    # interleave x loads with chunk-0 weight loads so the PE can start
    # after ~2 parallel transfers
    ring()(xt_t[0][:, :HT * scols[0]], inb[off["xt"]][:, :HT * scols[0]])
    wg_t = pools["wg"].tile([P, HT * CW], BF16, tag="wg")
    ring()(wg_t[:], inb[off["wg"]])
    for pr in range(1, npair):
        ring()(xt_t[pr][:, :HT * scols[pr]],
               inb[off["xt"] + pr][:, :HT * scols[pr]])
    wu_t = pools["wu"].tile([P, HT * CW], BF16, tag="wu")
    ring()(wu_t[:], inb[off["wu"]])
    wd_t = pools["wd"].tile([P, CHUNK * H], BF16, tag="wd")
    ring()(wd_t[:], inb[off["wd"]])

    for c in range(nch):
        if c > 0:
            wg_t = pools["wg"].tile([P, HT * CW], BF16, tag="wg")
            ring()(wg_t[:], inb[off["wg"] + c])
            wu_t = pools["wu"].tile([P, HT * CW], BF16, tag="wu")
            ring()(wu_t[:], inb[off["wu"] + c])
            wd_t = pools["wd"].tile([P, CHUNK * H], BF16, tag="wd")
            ring()(wd_t[:], inb[off["wd"] + c])
        at_t = pools["aT"].tile([P, CHUNK * 2 * S2], BF16, tag="aT")

        for it in range(CHUNK):
            ps_g = [pools["psgu"].tile([P, S2], F32, tag="ps",
                                       name=f"psg{pr}")
                    for pr in range(npair)]
            ps_u = [pools["psgu"].tile([P, S2], F32, tag="ps",
                                       name=f"psu{pr}")
                    for pr in range(npair)]
            for t in range(HT):
                col = t * CW + it * P
                for pr in range(npair):
                    nc.tensor.matmul(ps_g[pr][:, :scols[pr]],
                                     wg_t[:, col:col + P],
                                     xt_t[pr][:, t * scols[pr]:
                                               (t + 1) * scols[pr]],
                                     start=(t == 0), stop=(t == HT - 1))
            for t in range(HT):
                col = t * CW + it * P
                for pr in range(npair):
                    nc.tensor.matmul(ps_u[pr][:, :scols[pr]],
                                     wu_t[:, col:col + P],
                                     xt_t[pr][:, t * scols[pr]:
                                               (t + 1) * scols[pr]],
                                     start=(t == 0), stop=(t == HT - 1))
            for pr in range(npair):
                tmp = pools["tmp"].tile([P, S2], F32, tag="tmp")
                nc.scalar.activation(tmp[:, :scols[pr]],
                                     ps_g[pr][:, :scols[pr]], GELU)
                acol = it * sumsc + pre[pr]
                nc.vector.tensor_mul(at_t[:, acol:acol + scols[pr]],
                                     tmp[:, :scols[pr]],
                                     ps_u[pr][:, :scols[pr]])

        last = (c == nch - 1)
        for pr in range(npair):
            for ss in range(2 * sizes[pr]):
                sc = [pools["pssc"].tile([P, 512], F32, tag="sc",
                                         name=f"sc{hg}")
                      for hg in range(2)]
                for ci in range(CHUNK):
                    acol = ci * sumsc + pre[pr] + ss * P
                    for hg in range(2):
                        nc.tensor.matmul(
                            sc[hg][:], at_t[:, acol:acol + P],
                            wd_t[:, ci * H + hg * 512:ci * H + (hg + 1) * 512],
                            start=(ci == 0), stop=(ci == CHUNK - 1))
                for hg in range(2):
                    blk = slice((ss * 2 + hg) * 512, (ss * 2 + hg + 1) * 512)
                    if last:
                        # final chunk: write bf16 staging directly
                        if nch == 1:
                            nc.vector.tensor_copy(st_t[pr][:, blk], sc[hg][:])
                        else:
                            nc.vector.tensor_add(st_t[pr][:, blk],
                                                 out_sb[pr][:, blk], sc[hg][:])
                    elif c == 0:
                        nc.vector.tensor_copy(out_sb[pr][:, blk], sc[hg][:])
                    else:
                        nc.vector.tensor_add(out_sb[pr][:, blk],
                                             out_sb[pr][:, blk], sc[hg][:])
            if last:
                w = sizes[pr] * 4 * 512
                ring()(outb[off["out"] + pr][:, :w], st_t[pr][:, :w])


def _layout(groups):
    """Row offsets of each group's tensors in the input/output blobs."""
    offs, r, orow = [], 0, 0
    for sizes, ni in groups:
        npair = len(sizes)
        nch = ni // CHUNK
        offs.append({"xt": r, "wg": r + npair, "wu": r + npair + nch,
                     "wd": r + npair + 2 * nch, "out": orow})
        r += npair + 3 * nch
        orow += npair
    return offs, r, orow


def _build_kernel(groups):
    """groups: tuple of (pair_sizes_tuple, n_itiles) per slot."""
    nc = bacc.Bacc("TRN2", target_bir_lowering=False, debug=False,
                   num_devices=NCORES)
    offs, nin, nout = _layout(groups)
    inb = nc.dram_tensor("inb", [nin, P, 4096], BF16,
                         kind="ExternalInput").ap()
    outb = nc.dram_tensor("outb", [nout, P, 4096], BF16,
                          kind="ExternalOutput").ap()

    with tile.TileContext(nc) as tc, ExitStack() as ctx:
        pools = {
            "xt": ctx.enter_context(tc.tile_pool(name="xt", bufs=4)),
            "psgu": ctx.enter_context(
                tc.tile_pool(name="psgu", bufs=5, space="PSUM")),
            "pssc": ctx.enter_context(
                tc.tile_pool(name="pssc", bufs=3, space="PSUM")),
            "tmp": ctx.enter_context(tc.tile_pool(name="tmp", bufs=3)),
            "aT": ctx.enter_context(tc.tile_pool(name="aT", bufs=2)),
            "outsb": ctx.enter_context(tc.tile_pool(name="outsb", bufs=3)),
            "stage": ctx.enter_context(tc.tile_pool(name="stage", bufs=3)),
            "wg": ctx.enter_context(tc.tile_pool(name="wg", bufs=2)),
            "wu": ctx.enter_context(tc.tile_pool(name="wu", bufs=2)),
            "wd": ctx.enter_context(tc.tile_pool(name="wd", bufs=2)),
        }
        rng = {"i": 0}

        def ring():
            eng = nc.sync if rng["i"] % 2 == 0 else nc.scalar
            rng["i"] += 1
            return eng.dma_start

        for gi, (sizes, ni) in enumerate(groups):
            _group(nc, pools, ring, inb, offs[gi], outb, sizes, ni)

    nc.compile()
    return nc


_KERNEL_CACHE = {}


def _get_kernel(groups):
    if groups not in _KERNEL_CACHE:
        _KERNEL_CACHE[groups] = _build_kernel(groups)
    return _KERNEL_CACHE[groups]


def _routing(router_logits):
    """Replicate reference routing in numpy f32: softmax, top-2, renorm."""
    rl = np.asarray(router_logits, np.float32)
    m = rl.max(axis=-1, keepdims=True)
    ex = np.exp(rl - m, dtype=np.float32)
    rw = ex / ex.sum(axis=-1, keepdims=True)
    sel = np.argsort(-rw, axis=-1, kind="stable")[:, :TOP_K]
    w = np.take_along_axis(rw, sel, axis=-1)
    w = w / w.sum(axis=-1, keepdims=True)
    return sel, w.astype(np.float32)


def _pack_gu(w):
    """[H, Ic] f32 -> [nch, 128, HT*CHUNK*128] bf16 (chunk, p, t, it, ii)."""
    Ic = w.shape[1]
    nch = Ic // (CHUNK * P)
    return np.ascontiguousarray(
        w.astype(NPBF).reshape(HT, P, nch, CHUNK * P)
        .transpose(2, 1, 0, 3).reshape(nch, P, HT * CHUNK * P))


def _pack_d(wd):
    """[Ir, H] f32 -> [nch, 128, CHUNK*H] bf16 (chunk, p, ci, h)."""
    Ir = wd.shape[0]
    nch = Ir // (CHUNK * P)
    return np.ascontiguousarray(
        wd.astype(NPBF).reshape(nch, CHUNK, P, H)
        .transpose(0, 2, 1, 3).reshape(nch, P, CHUNK * H))


def _pack_x(xb):
    """[nb, S, H] f32 -> [128, HT*nb*S] bf16 (p, t, b, s)."""
    nb = xb.shape[0]
    return np.ascontiguousarray(
        xb.astype(NPBF).reshape(nb, S, HT, P)
        .transpose(3, 2, 0, 1).reshape(P, HT * nb * S))


def _unpack_out(r, nb):
    """[128, nb*2048] bf16 -> [nb, S, H] f32."""
    return (r[:, :nb * 2048].astype(np.float32)
            .reshape(P, nb, 2, 2, 512)            # p, b, sblk, hg, hh
            .transpose(1, 2, 0, 3, 4)
            .reshape(nb, S, H))


def kernel(x, router_logits, skill_gate, skill_up, skill_down,
           shared_gate, shared_up, shared_down):
    x = np.asarray(x, np.float32)

    sel, w = _routing(router_logits)
    lists = [[] for _ in range(E)]
    wmap = np.zeros((B, E), np.float32)
    for b in range(B):
        for k in range(TOP_K):
            e = int(sel[b, k])
            lists[e].append(b)
            wmap[b, e] = w[b, k]

    # decompose each expert's routed batches into full pairs + <=1 half pair
    fulls, halves = [], []            # (e, (b0, b1)) / (e, (b0,))
    for e in range(E):
        bs = lists[e]
        for i in range(0, len(bs) - 1, 2):
            fulls.append((e, (bs[i], bs[i + 1])))
        if len(bs) % 2:
            halves.append((e, (bs[-1],)))

    # same-expert two-pair groups: floor(count/8) per core, uniform
    by_e = {}
    for f in fulls:
        by_e.setdefault(f[0], []).append(f)
    g22_all = []
    for e in sorted(by_e):
        fl = by_e[e]
        while len(fl) >= 2:
            g22_all.append((e, fl.pop()[1] + fl.pop()[1]))
    n22 = len(g22_all) // NCORES
    # dissolve unused 22-groups back into single pairs
    rest = [(e, ent[0:2]) for e, ent in g22_all[n22 * NCORES:]] + \
           [(e, ent[2:4]) for e, ent in g22_all[n22 * NCORES:]]
    g22 = g22_all[:n22 * NCORES]
    singles = [(e, f) for e, fl in sorted(by_e.items()) for _, f in fl] + rest
    ns = len(singles) // NCORES
    own1 = singles[:ns * NCORES]
    tp_full = singles[ns * NCORES:]
    tp_half = halves

    cfg = (((2, 2), NIT),) * n22 + (((2,), NIT),) * ns \
        + (((2, 2), NIT),) \
        + (((2,), CHUNK),) * len(tp_full) + (((1,), CHUNK),) * len(tp_half)
    gi_shared = n22 + ns

    nc = _get_kernel(cfg)

    # pack weights once (bf16, SBUF layout)
    pg = [_pack_gu(np.asarray(skill_gate[e], np.float32)) for e in range(E)]
    pu = [_pack_gu(np.asarray(skill_up[e], np.float32)) for e in range(E)]
    pd = [_pack_d(np.asarray(skill_down[e], np.float32)) for e in range(E)]
    psg = _pack_gu(np.asarray(shared_gate, np.float32))
    psu = _pack_gu(np.asarray(shared_up, np.float32))
    psd = _pack_d(np.asarray(shared_down, np.float32))

    offs, nin, nout = _layout(cfg)
    in_maps = []
    core_groups = []
    for c in range(NCORES):
        own = [g22[c * n22 + j] for j in range(n22)] + \
              [own1[c * ns + j] for j in range(ns)]
        core_groups.append(own)
        blob = np.empty((nin, P, 4096), NPBF)
        for gi, (e, ent) in enumerate(own):
            o = offs[gi]
            npair = len(cfg[gi][0])
            for pr in range(npair):
                pk = _pack_x(x[list(ent[2 * pr:2 * pr + 2])])
                blob[o["xt"] + pr, :, :pk.shape[1]] = pk
            blob[o["wg"]:o["wg"] + 8] = pg[e]
            blob[o["wu"]:o["wu"] + 8] = pu[e]
            blob[o["wd"]:o["wd"] + 8] = pd[e]
        # shared group: batches 4c..4c+3, full I
        o = offs[gi_shared]
        blob[o["xt"]] = _pack_x(x[4 * c:4 * c + 2])
        blob[o["xt"] + 1] = _pack_x(x[4 * c + 2:4 * c + 4])
        blob[o["wg"]:o["wg"] + 8] = psg
        blob[o["wu"]:o["wu"] + 8] = psu
        blob[o["wd"]:o["wd"] + 8] = psd
        # tp slots: chunk c of expert e's packed weights is exactly this
        # core's i-slice [c*512, (c+1)*512)
        for tj, (e, ent) in enumerate(tp_full + tp_half):
            o = offs[gi_shared + 1 + tj]
            pk = _pack_x(x[list(ent)])
            blob[o["xt"], :, :pk.shape[1]] = pk
            blob[o["wg"]] = pg[e][c]
            blob[o["wu"]] = pu[e][c]
            blob[o["wd"]] = pd[e][c]
        in_maps.append({"inb": blob})

    res = run_bass_kernel_spmd(nc, in_maps, core_ids=list(range(NCORES)))
    kernel.last_exec_time_ns = res.exec_time_ns
    kernel.last_results = res
    kernel.last_nc = nc
    kernel.last_in_maps = in_maps

    out = np.zeros((B, S, H), np.float32)
    for c in range(NCORES):
        ob = res.results[c]["outb"]
        for gi, (e, ent) in enumerate(core_groups[c]):
            o = offs[gi]
            for pr in range(len(cfg[gi][0])):
                nb = cfg[gi][0][pr]
                arr = _unpack_out(ob[o["out"] + pr], nb)
                for j in range(nb):
                    b = ent[2 * pr + j]
                    out[b] += wmap[b, e] * arr[j]
        o = offs[gi_shared]
        for pr in range(2):
            arr = _unpack_out(ob[o["out"] + pr], 2)
            for j in range(2):
                out[4 * c + 2 * pr + j] += arr[j]
    for tj, (e, ent) in enumerate(tp_full + tp_half):
        o = offs[gi_shared + 1 + tj]
        nb = len(ent)
        acc = sum(_unpack_out(res.results[c]["outb"][o["out"]], nb)
                  for c in range(NCORES))
        for j in range(nb):
            out[ent[j]] += wmap[ent[j], e] * acc[j]
    return out


# revision 12
# speedup vs baseline: 14.6429x; 1.0017x over previous
"""MoE GemmaMLP (top-2 of 8 experts + shared expert) on 8 trn2 NeuronCores.

v4: bf16, host-packed contiguous DMA, shared expert folded in as a
data-parallel group, zero-padding load balance with half-pair slots.

Sharding: expert-parallel with load balancing.  The host computes top-2
routing from router_logits and decomposes each expert's routed batches into
full pairs (2 batches, 512 moving columns) and at most one half pair
(1 batch, 256 moving columns).  These are packed into weight-stream groups
so that every core gets the identical (SPMD-uniform) slot config with no
padded dummy batches:
  - n22 two-pair expert groups + ns single-pair expert groups (full I),
  - one two-pair shared-expert group (data-parallel: core c owns batches
    4c..4c+3, full I),
  - leftover full/half pairs as tensor-parallel slots (I/8 slice per core,
    host-reduced across cores).

Everything the device touches is bf16 and packed on the host into the exact
SBUF tile layout, so every DMA is a plain contiguous [128, <=4096] copy
(128 descriptors x <=8KB) — minimizing HWDGE issue-path pressure, the
dominant HW overhead.  DMAs alternate between the SP and ACT HWDGE rings;
the first chunk's weights are interleaved with the x loads so the PE can
start after ~2 transfers.

Per group: expert weights stream from HBM exactly once, chunked 4 i-tiles
at a time; gate/up matmuls (stationary = weight tile, moving = x columns)
fill PSUM, gelu*up produces a^T in bf16, and the down projection contracts
each chunk with hg-interleaved PSUM chains (stationary a^T block loaded
once for both h-halves), accumulating into per-pair f32 SBUF tiles; the
last chunk's accumulation writes a bf16 staging tile directly, which is
DMA'd per pair as soon as it completes to a packed output blob the host
unpacks, weights (routing), and reduces.
"""

import numpy as np
from contextlib import ExitStack

import concourse.bass as bass
import concourse.mybir as mybir
import concourse.tile as tile
from concourse import bacc
from concourse.bass_utils import run_bass_kernel_spmd

B, S, H, I, E = 32, 256, 1024, 4096, 8
TOP_K = 2
NCORES = 8
P = 128
HT = H // P               # 8 h-tiles
S2 = 2 * S                # 512 moving columns (one full pair)
CHUNK = 4                 # i-tiles per weight-stream / down-accum chunk
NIT = I // P              # 32 i-tiles for a full expert

F32 = mybir.dt.float32
BF16 = mybir.dt.bfloat16
NPBF = mybir.dt.np(BF16)
GELU = mybir.ActivationFunctionType.Gelu_apprx_tanh


def _group(nc, pools, ring, inb, off, outb, sizes, ni):
    """One weight-stream group: pairs of `sizes` batches (2=full, 1=half)
    sharing one expert's weights over `ni` i-tiles."""
    npair = len(sizes)
    scols = [s * S for s in sizes]
    sumsc = sum(scols)
    pre = [sum(scols[:i]) for i in range(npair)]
    nch = ni // CHUNK
    CW = CHUNK * P        # 512 i-cols per chunk

    xt_t = [pools["xt"].tile([P, HT * S2], BF16, tag="xt", name=f"xt{pr}")
            for pr in range(npair)]
    out_sb = [pools["outsb"].tile([P, 8 * 512], F32, tag="outsb",
                                  name=f"osb{pr}")
              for pr in range(npair)] if nch > 1 else [None] * npair
    st_t = [pools["stage"].tile([P, 8 * 512], BF16, tag="stage",
                                name=f"st{pr}")
            for pr in range(npair)]

    # interleave x loads with chunk-0 weight loads so the PE can start
    # after ~2 parallel transfers
    ring()(xt_t[0][:, :HT * scols[0]], inb[off["xt"]][:, :HT * scols[0]])
    wg_t = pools["wg"].tile([P, HT * CW], BF16, tag="wg")
    ring()(wg_t[:], inb[off["wg"]])
    for pr in range(1, npair):
        ring()(xt_t[pr][:, :HT * scols[pr]],
               inb[off["xt"] + pr][:, :HT * scols[pr]])
    wu_t = pools["wu"].tile([P, HT * CW], BF16, tag="wu")
    ring()(wu_t[:], inb[off["wu"]])
    wd_t = pools["wd"].tile([P, CHUNK * H], BF16, tag="wd")
    ring()(wd_t[:], inb[off["wd"]])

    for c in range(nch):
        if c > 0:
            wg_t = pools["wg"].tile([P, HT * CW], BF16, tag="wg")
            ring()(wg_t[:], inb[off["wg"] + c])
            wu_t = pools["wu"].tile([P, HT * CW], BF16, tag="wu")
            ring()(wu_t[:], inb[off["wu"] + c])
            wd_t = pools["wd"].tile([P, CHUNK * H], BF16, tag="wd")
            ring()(wd_t[:], inb[off["wd"] + c])
        at_t = pools["aT"].tile([P, CHUNK * 2 * S2], BF16, tag="aT")

        for it in range(CHUNK):
            ps_g = [pools["psgu"].tile([P, S2], F32, tag="ps",
                                       name=f"psg{pr}")
                    for pr in range(npair)]
            ps_u = [pools["psgu"].tile([P, S2], F32, tag="ps",
                                       name=f"psu{pr}")
                    for pr in range(npair)]
            for t in range(HT):
                col = t * CW + it * P
                for pr in range(npair):
                    nc.tensor.matmul(ps_g[pr][:, :scols[pr]],
                                     wg_t[:, col:col + P],
                                     xt_t[pr][:, t * scols[pr]:
                                               (t + 1) * scols[pr]],
                                     start=(t == 0), stop=(t == HT - 1))
            for t in range(HT):
                col = t * CW + it * P
                for pr in range(npair):
                    nc.tensor.matmul(ps_u[pr][:, :scols[pr]],
                                     wu_t[:, col:col + P],
                                     xt_t[pr][:, t * scols[pr]:
                                               (t + 1) * scols[pr]],
                                     start=(t == 0), stop=(t == HT - 1))
            for pr in range(npair):
                tmp = pools["tmp"].tile([P, S2], F32, tag="tmp")
                nc.scalar.activation(tmp[:, :scols[pr]],
                                     ps_g[pr][:, :scols[pr]], GELU)
                acol = it * sumsc + pre[pr]
                nc.vector.tensor_mul(at_t[:, acol:acol + scols[pr]],
                                     tmp[:, :scols[pr]],
                                     ps_u[pr][:, :scols[pr]])

        last = (c == nch - 1)
        for pr in range(npair):
            for ss in range(2 * sizes[pr]):
                sc = [pools["pssc"].tile([P, 512], F32, tag="sc",
                                         name=f"sc{hg}")
                      for hg in range(2)]
                for ci in range(CHUNK):
                    acol = ci * sumsc + pre[pr] + ss * P
                    for hg in range(2):
                        nc.tensor.matmul(
                            sc[hg][:], at_t[:, acol:acol + P],
                            wd_t[:, ci * H + hg * 512:ci * H + (hg + 1) * 512],
                            start=(ci == 0), stop=(ci == CHUNK - 1))
                for hg in range(2):
                    blk = slice((ss * 2 + hg) * 512, (ss * 2 + hg + 1) * 512)
                    if last:
                        # final chunk: write bf16 staging directly
                        if nch == 1:
                            nc.vector.tensor_copy(st_t[pr][:, blk], sc[hg][:])
                        else:
                            nc.vector.tensor_add(st_t[pr][:, blk],
                                                 out_sb[pr][:, blk], sc[hg][:])
                    elif c == 0:
                        nc.vector.tensor_copy(out_sb[pr][:, blk], sc[hg][:])
                    else:
                        nc.vector.tensor_add(out_sb[pr][:, blk],
                                             out_sb[pr][:, blk], sc[hg][:])
            if last:
                w = sizes[pr] * 4 * 512
                ring()(outb[off["out"] + pr][:, :w], st_t[pr][:, :w])


def _layout(groups):
    """Row offsets of each group's tensors in the input/output blobs."""
    offs, r, orow = [], 0, 0
    for sizes, ni in groups:
        npair = len(sizes)
        nch = ni // CHUNK
        offs.append({"xt": r, "wg": r + npair, "wu": r + npair + nch,
                     "wd": r + npair + 2 * nch, "out": orow})
        r += npair + 3 * nch
        orow += npair
    return offs, r, orow


def _build_kernel(groups):
    """groups: tuple of (pair_sizes_tuple, n_itiles) per slot."""
    nc = bacc.Bacc("TRN2", target_bir_lowering=False, debug=False,
                   num_devices=NCORES)
    offs, nin, nout = _layout(groups)
    inb = nc.dram_tensor("inb", [nin, P, 4096], BF16,
                         kind="ExternalInput").ap()
    outb = nc.dram_tensor("outb", [nout, P, 4096], BF16,
                          kind="ExternalOutput").ap()

    with tile.TileContext(nc) as tc, ExitStack() as ctx:
        pools = {
            "xt": ctx.enter_context(tc.tile_pool(name="xt", bufs=4)),
            "psgu": ctx.enter_context(
                tc.tile_pool(name="psgu", bufs=5, space="PSUM")),
            "pssc": ctx.enter_context(
                tc.tile_pool(name="pssc", bufs=3, space="PSUM")),
            "tmp": ctx.enter_context(tc.tile_pool(name="tmp", bufs=3)),
            "aT": ctx.enter_context(tc.tile_pool(name="aT", bufs=2)),
            "outsb": ctx.enter_context(tc.tile_pool(name="outsb", bufs=3)),
            "stage": ctx.enter_context(tc.tile_pool(name="stage", bufs=3)),
            "wg": ctx.enter_context(tc.tile_pool(name="wg", bufs=2)),
            "wu": ctx.enter_context(tc.tile_pool(name="wu", bufs=2)),
            "wd": ctx.enter_context(tc.tile_pool(name="wd", bufs=2)),
        }
        rng = {"i": 0}

        def ring():
            eng = nc.sync if rng["i"] % 2 == 0 else nc.scalar
            rng["i"] += 1
            return eng.dma_start

        for gi, (sizes, ni) in enumerate(groups):
            _group(nc, pools, ring, inb, offs[gi], outb, sizes, ni)

    nc.compile()
    return nc


_KERNEL_CACHE = {}


def _get_kernel(groups):
    if groups not in _KERNEL_CACHE:
        _KERNEL_CACHE[groups] = _build_kernel(groups)
    return _KERNEL_CACHE[groups]


def _routing(router_logits):
    """Replicate reference routing in numpy f32: softmax, top-2, renorm."""
    rl = np.asarray(router_logits, np.float32)
    m = rl.max(axis=-1, keepdims=True)
    ex = np.exp(rl - m, dtype=np.float32)
    rw = ex / ex.sum(axis=-1, keepdims=True)
    sel = np.argsort(-rw, axis=-1, kind="stable")[:, :TOP_K]
    w = np.take_along_axis(rw, sel, axis=-1)
    w = w / w.sum(axis=-1, keepdims=True)
    return sel, w.astype(np.float32)


def _pack_gu(w):
    """[H, Ic] f32 -> [nch, 128, HT*CHUNK*128] bf16 (chunk, p, t, it, ii)."""
    Ic = w.shape[1]
    nch = Ic // (CHUNK * P)
    return np.ascontiguousarray(
        w.astype(NPBF).reshape(HT, P, nch, CHUNK * P)
        .transpose(2, 1, 0, 3).reshape(nch, P, HT * CHUNK * P))


def _pack_d(wd):
    """[Ir, H] f32 -> [nch, 128, CHUNK*H] bf16 (chunk, p, ci, h)."""
    Ir = wd.shape[0]
    nch = Ir // (CHUNK * P)
    return np.ascontiguousarray(
        wd.astype(NPBF).reshape(nch, CHUNK, P, H)
        .transpose(0, 2, 1, 3).reshape(nch, P, CHUNK * H))


def _pack_x(xb):
    """[nb, S, H] f32 -> [128, HT*nb*S] bf16 (p, t, b, s)."""
    nb = xb.shape[0]
    return np.ascontiguousarray(
        xb.astype(NPBF).reshape(nb, S, HT, P)
        .transpose(3, 2, 0, 1).reshape(P, HT * nb * S))


def _unpack_out(r, nb):
    """[128, nb*2048] bf16 -> [nb, S, H] f32."""
    return (r[:, :nb * 2048].astype(np.float32)
            .reshape(P, nb, 2, 2, 512)            # p, b, sblk, hg, hh
            .transpose(1, 2, 0, 3, 4)
            .reshape(nb, S, H))


def kernel(x, router_logits, skill_gate, skill_up, skill_down,
           shared_gate, shared_up, shared_down):
    x = np.asarray(x, np.float32)

    sel, w = _routing(router_logits)
    lists = [[] for _ in range(E)]
    wmap = np.zeros((B, E), np.float32)
    for b in range(B):
        for k in range(TOP_K):
            e = int(sel[b, k])
            lists[e].append(b)
            wmap[b, e] = w[b, k]

    # decompose each expert's routed batches into full pairs + <=1 half pair
    fulls, halves = [], []            # (e, (b0, b1)) / (e, (b0,))
    for e in range(E):
        bs = lists[e]
        for i in range(0, len(bs) - 1, 2):
            fulls.append((e, (bs[i], bs[i + 1])))
        if len(bs) % 2:
            halves.append((e, (bs[-1],)))

    # same-expert two-pair groups: floor(count/8) per core, uniform
    by_e = {}
    for f in fulls:
        by_e.setdefault(f[0], []).append(f)
    g22_all = []
    for e in sorted(by_e):
        fl = by_e[e]
        while len(fl) >= 2:
            g22_all.append((e, fl.pop()[1] + fl.pop()[1]))
    n22 = len(g22_all) // NCORES
    # dissolve unused 22-groups back into single pairs
    rest = [(e, ent[0:2]) for e, ent in g22_all[n22 * NCORES:]] + \
           [(e, ent[2:4]) for e, ent in g22_all[n22 * NCORES:]]
    g22 = g22_all[:n22 * NCORES]
    singles = [(e, f) for e, fl in sorted(by_e.items()) for _, f in fl] + rest
    ns = len(singles) // NCORES
    own1 = singles[:ns * NCORES]
    tp_full = singles[ns * NCORES:]
    tp_half = halves

    # slot list, identical shape sequence on every core.  kind is one of
    # "own" (per-core expert group), "shared", "tp" (replicated pair,
    # I/8 slice per core).  Half-pair tp slots trail to minimize the tail.
    slots = []
    for j in range(n22):
        slots.append(("own", None, None, "g22", j))
    for j in range(ns):
        slots.append(("own", None, None, "own1", j))
    slots.append(("shared", None, None))
    slots += [("tp", e, ent) for e, ent in tp_full + tp_half]

    cfg, kinds = [], []
    for s in slots:
        if s[0] == "tp":
            cfg.append(((len(s[2]),), CHUNK))
        else:
            src = s[3] if s[0] == "own" else None
            cfg.append(((2, 2) if (s[0] == "shared" or src == "g22")
                        else (2,), NIT))
    cfg = tuple(cfg)

    nc = _get_kernel(cfg)

    # pack weights once (bf16, SBUF layout)
    pg = [_pack_gu(np.asarray(skill_gate[e], np.float32)) for e in range(E)]
    pu = [_pack_gu(np.asarray(skill_up[e], np.float32)) for e in range(E)]
    pd = [_pack_d(np.asarray(skill_down[e], np.float32)) for e in range(E)]
    psg = _pack_gu(np.asarray(shared_gate, np.float32))
    psu = _pack_gu(np.asarray(shared_up, np.float32))
    psd = _pack_d(np.asarray(shared_down, np.float32))

    offs, nin, nout = _layout(cfg)
    in_maps = []
    core_ent = []       # per core, per slot: (e, batch tuple)
    for c in range(NCORES):
        blob = np.empty((nin, P, 4096), NPBF)
        ents = []
        for gi, s in enumerate(slots):
            o = offs[gi]
            if s[0] == "own":
                e, ent = (g22[c * n22 + s[4]] if s[3] == "g22"
                          else own1[c * ns + s[4]])
                wg_, wu_, wd_ = pg[e], pu[e], pd[e]
            elif s[0] == "shared":
                # data-parallel: core c owns batches 4c..4c+3, full I
                e, ent = None, tuple(range(4 * c, 4 * c + 4))
                wg_, wu_, wd_ = psg, psu, psd
            else:
                # tp slot: chunk c of expert e's packed weights is exactly
                # this core's i-slice [c*512, (c+1)*512)
                e, ent = s[1], tuple(s[2])
                wg_, wu_, wd_ = (pg[e][c:c + 1], pu[e][c:c + 1],
                                 pd[e][c:c + 1])
            ents.append((e, ent))
            pos = 0
            for pr, nb in enumerate(cfg[gi][0]):
                pk = _pack_x(x[list(ent[pos:pos + nb])])
                blob[o["xt"] + pr, :, :pk.shape[1]] = pk
                pos += nb
            nch = cfg[gi][1] // CHUNK
            blob[o["wg"]:o["wg"] + nch] = wg_
            blob[o["wu"]:o["wu"] + nch] = wu_
            blob[o["wd"]:o["wd"] + nch] = wd_
        core_ent.append(ents)
        in_maps.append({"inb": blob})

    res = run_bass_kernel_spmd(nc, in_maps, core_ids=list(range(NCORES)))
    kernel.last_exec_time_ns = res.exec_time_ns
    kernel.last_results = res
    kernel.last_nc = nc
    kernel.last_in_maps = in_maps

    out = np.zeros((B, S, H), np.float32)
    for gi, s in enumerate(slots):
        o = offs[gi]["out"]
        sizes = cfg[gi][0]
        if s[0] == "tp":
            # partial (I/8) sums — reduce across cores, then weight
            e, ent = s[1], tuple(s[2])
            nb = len(ent)
            acc = sum(_unpack_out(res.results[c]["outb"][o], nb)
                      for c in range(NCORES))
            for j in range(nb):
                out[ent[j]] += wmap[ent[j], e] * acc[j]
        else:
            for c in range(NCORES):
                e, ent = core_ent[c][gi]
                ob = res.results[c]["outb"]
                pos = 0
                for pr, nb in enumerate(sizes):
                    arr = _unpack_out(ob[o + pr], nb)
                    for j in range(nb):
                        b = ent[pos + j]
                        out[b] += (arr[j] if s[0] == "shared"
                                   else wmap[b, e] * arr[j])
                    pos += nb
    return out
